# revision 9
# baseline (speedup 1.0000x reference)
"""Deductron kernel for Trainium2, 8 NeuronCores, time-sharded.

Math (matching the reference):
    h = sigmoid(W1 @ x + B1); left, right = h[:128], h[128:]
    a_t = left_t * right_t; b_t = 1 - left_t
    u_0 = 0; u_t = a_{t-1} * u_{t-1} + b_{t-1}   (z[:, t] = u_t)
    out = 1 - sigmoid(W2 @ z + B2) = sigmoid(-(W2 @ z + B2))

Sharding: the 65536-frame time axis is split into 8 chunks of 8192. Each core
also receives a 512-frame left halo. Because a_t = sigmoid(.)*sigmoid(.) < 1
and the product of 512 consecutive a's underflows to exactly 0 in fp32, the
recurrence state forgets its initial condition within the halo, so no
cross-core state exchange is needed. Core 0 has no real halo: its input is
zero-padded and a per-core input vector bscale (0 for core 0, 1 otherwise)
multiplies the halo's b values so the scan state stays exactly 0 until the
owned region starts (u_0 = 0 exactly).

On-core recurrence: the Vector engine's tensor_tensor_scan instruction
computes state = a_t*state + b_t natively along the free axis; chunk-level
scans are chained by a 128x1 carry column.
"""

import sys

for _p in ("/opt/trn_rl_repo", "/opt/pypackages"):
    if _p not in sys.path:
        sys.path.append(_p)

import numpy as np

# Problem constants (hardcoded per contract).
INPUT_LEN = 512
N_MEM = 128  # memory dim (recurrence state width) = one partition tile
OUT_LEN = 256
T_TOTAL = 65536
N_CORES = 8
T_LOC = T_TOTAL // N_CORES  # 8192 owned frames per core
HALO = 512                  # washout halo; prod(a) over 512 steps == 0 in fp32
TW = 512                    # column tile width (max fp32 matmul moving dim)
W_IN = HALO + T_LOC         # per-core input width (8704)
NT = W_IN // TW             # 17 column tiles (tile 0 is pure halo)


def _build_nc(t_loc=T_LOC, halo=HALO, tw=TW):
    import concourse.tile as tile
    from concourse import bacc, mybir
    from concourse.masks import make_identity
    from contextlib import ExitStack

    F32 = mybir.dt.float32
    F32R = mybir.dt.float32r
    BF16 = mybir.dt.bfloat16
    SIG = mybir.ActivationFunctionType.Sigmoid
    MUL = mybir.AluOpType.mult
    ADD = mybir.AluOpType.add

    w_in = halo + t_loc
    nt = w_in // tw
    assert w_in % tw == 0 and halo == tw

    nc = bacc.Bacc()
    x = nc.dram_tensor("x", [INPUT_LEN, w_in], F32, kind="ExternalInput")
    w1 = nc.dram_tensor("w1", [2 * N_MEM, INPUT_LEN], F32, kind="ExternalInput")
    b1 = nc.dram_tensor("b1", [2 * N_MEM, 1], F32, kind="ExternalInput")
    w2 = nc.dram_tensor("w2", [OUT_LEN, N_MEM], F32, kind="ExternalInput")
    b2 = nc.dram_tensor("b2", [OUT_LEN, 1], F32, kind="ExternalInput")
    bscale = nc.dram_tensor("bscale", [N_MEM, 1], F32, kind="ExternalInput")
    out = nc.dram_tensor("out", [OUT_LEN, t_loc], F32, kind="ExternalOutput")

    with ExitStack() as ctx:
        tc = ctx.enter_context(tile.TileContext(nc))
        singles = ctx.enter_context(tc.tile_pool(name="singles", bufs=1))
        xpool = ctx.enter_context(tc.tile_pool(name="xpool", bufs=3))
        hpool = ctx.enter_context(tc.tile_pool(name="hpool", bufs=3))
        opool = ctx.enter_context(tc.tile_pool(name="opool", bufs=3))
        psA = ctx.enter_context(tc.tile_pool(name="psA", bufs=2, space="PSUM"))
        psB = ctx.enter_context(tc.tile_pool(name="psB", bufs=2, space="PSUM"))

        # Persistent full-width recurrence buffers. a_buf/b_buf are written at
        # a +1 column offset (a_buf[:, p] = a at input column p-1) so the scan
        # output z[:, p] = u at column p directly.
        a_buf = singles.tile([N_MEM, w_in + 1], F32)
        b_buf = singles.tile([N_MEM, w_in + 1], F32)
        z_buf = singles.tile([N_MEM, w_in], BF16)

        # ---- weights / biases / constants ----
        w1_sb = singles.tile([128, 2, INPUT_LEN], F32)
        nc.sync.dma_start(out=w1_sb, in_=w1[:].rearrange("(m p) k -> p m k", p=128))
        w2_sb = singles.tile([128, 2, N_MEM], F32)
        nc.sync.dma_start(out=w2_sb, in_=w2[:].rearrange("(m p) k -> p m k", p=128))
        b1_sb = singles.tile([128, 2, 1], F32)
        nc.sync.dma_start(out=b1_sb, in_=b1[:].rearrange("(m p) o -> p m o", p=128))
        b2_sb = singles.tile([128, 2, 1], F32)
        nc.sync.dma_start(out=b2_sb, in_=b2[:].rearrange("(m p) o -> p m o", p=128))
        bs_sb = singles.tile([128, 1], F32)
        nc.sync.dma_start(out=bs_sb, in_=bscale[:])

        negb2 = singles.tile([128, 2, 1], F32)
        nc.vector.tensor_scalar_mul(negb2, b2_sb, -1.0)

        ident = singles.tile([128, 128], F32)
        make_identity(nc, ident)

        # Transposed stationary operands: w1t[p=k_in, k_out, m], w2t[p=k, m, j]
        w1t = singles.tile([128, 4, 2 * N_MEM], BF16)
        for m in range(2):
            for k in range(4):
                ps = psB.tile([128, 128], F32, tag="o0")
                nc.tensor.transpose(ps, w1_sb[:, m, k * 128:(k + 1) * 128], ident)
                nc.vector.tensor_copy(out=w1t[:, k, m * 128:(m + 1) * 128], in_=ps)
        w2t = singles.tile([128, 2, N_MEM], BF16)
        for m in range(2):
            ps = psB.tile([128, 128], F32, tag="o1")
            nc.tensor.transpose(ps, w2_sb[:, m, :], ident)
            nc.vector.tensor_copy(out=w2t[:, m, :], in_=ps)

        nc.vector.memset(a_buf[:, 0:1], 0.0)
        nc.vector.memset(b_buf[:, 0:1], 0.0)

        xr = x[:].rearrange("(k p) t -> p k t", p=128)       # (128, 4, w_in)
        outr = out[:].rearrange("(m p) t -> p m t", p=128)   # (128, 2, t_loc)

        for j in range(nt):
            c0 = j * tw
            # ---- phase A: h-GEMM + activations ----
            xt = xpool.tile([128, 4, tw], F32)
            nc.sync.dma_start(out=xt, in_=xr[:, :, c0:c0 + tw])
            xb = xpool.tile([128, 4, tw], BF16)
            nc.gpsimd.tensor_copy(out=xb, in_=xt)
            g0 = psA.tile([128, tw], F32)
            g1 = psA.tile([128, tw], F32)
            for k in range(4):
                nc.tensor.matmul(
                    g0, lhsT=w1t[:, k, 0:128], rhs=xb[:, k, :],
                    start=(k == 0), stop=(k == 3))
            for k in range(4):
                nc.tensor.matmul(
                    g1, lhsT=w1t[:, k, 128:256], rhs=xb[:, k, :],
                    start=(k == 0), stop=(k == 3))
            left = hpool.tile([128, tw], F32)
            right = hpool.tile([128, tw], F32)
            nc.scalar.activation(left, g0, SIG, bias=b1_sb[:, 0, :], scale=1.0)
            nc.scalar.activation(right, g1, SIG, bias=b1_sb[:, 1, :], scale=1.0)
            # b = 1 - left, on GpSimd (idle otherwise): (left * -1) + 1
            nc.gpsimd.tensor_scalar(
                out=b_buf[:, c0 + 1:c0 + 1 + tw], in0=left,
                scalar1=-1.0, scalar2=1.0, op0=MUL, op1=ADD)
            nc.vector.tensor_mul(a_buf[:, c0 + 1:c0 + 1 + tw], left, right)

            if j == 0:
                # Halo b *= bscale (covers cols [0, halo]; col 0 is the memset)
                nc.vector.tensor_scalar(
                    out=b_buf[:, 0:halo + 1], in0=b_buf[:, 0:halo + 1],
                    scalar1=bs_sb[:, 0:1], scalar2=None, op0=MUL)

            # ---- phase B: recurrence scan over this tile ----
            init = 0.0 if j == 0 else z_buf[:, c0 - 1:c0]
            nc.vector.tensor_tensor_scan(
                out=z_buf[:, c0:c0 + tw],
                data0=a_buf[:, c0:c0 + tw],
                data1=b_buf[:, c0:c0 + tw],
                initial=init, op0=MUL, op1=ADD)

            # ---- phase C: output GEMM + activation + store ----
            if j >= 1:
                zr = z_buf[:, c0:c0 + tw]
                o0 = psB.tile([128, tw], F32)
                o1 = psB.tile([128, tw], F32)
                nc.tensor.matmul(o0, lhsT=w2t[:, 0, :], rhs=zr,
                                 start=True, stop=True)
                nc.tensor.matmul(o1, lhsT=w2t[:, 1, :], rhs=zr,
                                 start=True, stop=True)
                ot = opool.tile([128, 2, tw], F32)
                nc.scalar.activation(ot[:, 0, :], o0, SIG,
                                     bias=negb2[:, 0, :], scale=-1.0)
                nc.scalar.activation(ot[:, 1, :], o1, SIG,
                                     bias=negb2[:, 1, :], scale=-1.0)
                nc.sync.dma_start(out=outr[:, :, c0 - halo:c0 - halo + tw], in_=ot)

    nc.finalize()
    return nc


def _make_in_maps(inputs, W1, B1, W2, B2, t_loc=T_LOC, halo=HALO, n_cores=N_CORES):
    inputs = np.ascontiguousarray(np.asarray(inputs, dtype=np.float32))
    W1 = np.ascontiguousarray(np.asarray(W1, dtype=np.float32))
    B1 = np.ascontiguousarray(np.asarray(B1, dtype=np.float32))
    W2 = np.ascontiguousarray(np.asarray(W2, dtype=np.float32))
    B2 = np.ascontiguousarray(np.asarray(B2, dtype=np.float32))
    in_maps = []
    for i in range(n_cores):
        s = i * t_loc
        lo = s - halo
        if lo < 0:
            xs = np.concatenate(
                [np.zeros((INPUT_LEN, -lo), np.float32), inputs[:, :s + t_loc]],
                axis=1)
        else:
            xs = inputs[:, lo:s + t_loc]
        bs = np.full((N_MEM, 1), 0.0 if i == 0 else 1.0, np.float32)
        in_maps.append({
            "x": np.ascontiguousarray(xs),
            "w1": W1, "b1": B1, "w2": W2, "b2": B2, "bscale": bs,
        })
    return in_maps


def _run(inputs, W1, B1, W2, B2, trace=False, **kw):
    from concourse.bass_utils import run_bass_kernel_spmd

    nc = _build_nc()
    in_maps = _make_in_maps(inputs, W1, B1, W2, B2)
    res = run_bass_kernel_spmd(nc, in_maps, list(range(N_CORES)), trace=trace, **kw)
    full = np.concatenate([r["out"] for r in res.results], axis=1)
    return full, res


def kernel(inputs, W1, B1, W2, B2):
    full, _ = _run(inputs, W1, B1, W2, B2, trace=False)
    return full.astype(np.float32, copy=False)


# revision 10
# speedup vs baseline: 2.4035x; 2.4035x over previous
"""Deductron kernel for Trainium2, 8 NeuronCores, time-sharded.

Math (matching the reference):
    h = sigmoid(W1 @ x + B1); left, right = h[:128], h[128:]
    a_t = left_t * right_t; b_t = 1 - left_t
    u_0 = 0; u_t = a_{t-1} * u_{t-1} + b_{t-1}   (z[:, t] = u_t)
    out = 1 - sigmoid(W2 @ z + B2) = sigmoid(-(W2 @ z + B2))

Sharding: the 65536-frame time axis is split into 8 chunks of 8192. Each core
also receives a 512-frame left halo. Because a_t = sigmoid(.)*sigmoid(.) < 1
and the product of 512 consecutive a's underflows to exactly 0 in fp32, the
recurrence state forgets its initial condition within the halo, so no
cross-core state exchange is needed. Core 0 has no real halo: its input is
zero-padded and a per-core input vector bscale (0 for core 0, 1 otherwise)
multiplies the halo's b values so the scan state stays exactly 0 until the
owned region starts (u_0 = 0 exactly).

The host pre-casts x to bf16 (halves input DMA) and pre-transposes the
weights into the PE's stationary layout. On-core recurrence: the Vector
engine's tensor_tensor_scan instruction computes state = a_t*state + b_t
natively along the free axis; chunk scans are chained by a 128x1 carry.
"""

import sys

for _p in ("/opt/trn_rl_repo", "/opt/pypackages"):
    if _p not in sys.path:
        sys.path.append(_p)

import numpy as np
import ml_dtypes

# Problem constants (hardcoded per contract).
INPUT_LEN = 512
N_MEM = 128  # memory dim (recurrence state width) = one partition tile
OUT_LEN = 256
T_TOTAL = 65536
N_CORES = 8
T_LOC = T_TOTAL // N_CORES  # 8192 owned frames per core
HALO = 512                  # washout halo; prod(a) over 512 steps == 0 in fp32
TW = 512                    # column tile width (one PSUM bank of fp32)
W_IN = HALO + T_LOC         # per-core input width (8704)
NT = W_IN // TW             # 17 column tiles (tile 0 is pure halo)

BF16_NP = ml_dtypes.bfloat16


def _build_nc(t_loc=T_LOC, halo=HALO, tw=TW):
    import concourse.tile as tile
    from concourse import bacc, mybir
    from contextlib import ExitStack

    F32 = mybir.dt.float32
    BF16 = mybir.dt.bfloat16
    SIG = mybir.ActivationFunctionType.Sigmoid
    MUL = mybir.AluOpType.mult
    ADD = mybir.AluOpType.add

    w_in = halo + t_loc
    nt = w_in // tw
    assert w_in % tw == 0 and halo == tw

    nc = bacc.Bacc()
    x = nc.dram_tensor("x", [INPUT_LEN, w_in], BF16, kind="ExternalInput")
    w1t = nc.dram_tensor("w1t", [INPUT_LEN, 2 * N_MEM], BF16, kind="ExternalInput")
    w2t = nc.dram_tensor("w2t", [N_MEM, OUT_LEN], BF16, kind="ExternalInput")
    b1 = nc.dram_tensor("b1", [2 * N_MEM, 1], F32, kind="ExternalInput")
    negb1t = nc.dram_tensor("negb1t", [N_MEM, 1], F32, kind="ExternalInput")
    negb2 = nc.dram_tensor("negb2", [OUT_LEN, 1], F32, kind="ExternalInput")
    bscale = nc.dram_tensor("bscale", [N_MEM, 1], F32, kind="ExternalInput")
    out = nc.dram_tensor("out", [OUT_LEN, t_loc], F32, kind="ExternalOutput")

    with ExitStack() as ctx:
        tc = ctx.enter_context(tile.TileContext(nc))
        singles = ctx.enter_context(tc.tile_pool(name="singles", bufs=1))
        xpool = ctx.enter_context(tc.tile_pool(name="xpool", bufs=3))
        hpool = ctx.enter_context(tc.tile_pool(name="hpool", bufs=3))
        opool = ctx.enter_context(tc.tile_pool(name="opool", bufs=3))
        psA = ctx.enter_context(tc.tile_pool(name="psA", bufs=2, space="PSUM"))
        psB = ctx.enter_context(tc.tile_pool(name="psB", bufs=2, space="PSUM"))

        # Persistent full-width recurrence buffers. a_buf/b_buf are written at
        # a +1 column offset (a_buf[:, p] = a at input column p-1) so the scan
        # output z[:, p] = u at column p directly.
        a_buf = singles.tile([N_MEM, w_in + 1], F32)
        b_buf = singles.tile([N_MEM, w_in + 1], F32)
        z_buf = singles.tile([N_MEM, w_in], BF16)

        # ---- weights / biases (host provides transposed layouts) ----
        w1t_sb = singles.tile([128, 4, 2 * N_MEM], BF16)
        nc.sync.dma_start(out=w1t_sb,
                          in_=w1t[:].rearrange("(k p) m -> p k m", p=128))
        w2t_sb = singles.tile([128, 2, N_MEM], BF16)
        nc.sync.dma_start(out=w2t_sb,
                          in_=w2t[:].rearrange("p (m j) -> p m j", m=2))
        b1_sb = singles.tile([128, 2, 1], F32)
        nc.sync.dma_start(out=b1_sb, in_=b1[:].rearrange("(m p) o -> p m o", p=128))
        negb1t_sb = singles.tile([128, 1], F32)
        nc.sync.dma_start(out=negb1t_sb, in_=negb1t[:])
        negb2_sb = singles.tile([128, 2, 1], F32)
        nc.sync.dma_start(out=negb2_sb,
                          in_=negb2[:].rearrange("(m p) o -> p m o", p=128))
        bs_sb = singles.tile([128, 1], F32)
        nc.sync.dma_start(out=bs_sb, in_=bscale[:])

        nc.vector.memset(a_buf[:, 0:1], 0.0)
        nc.vector.memset(b_buf[:, 0:1], 0.0)

        xr = x[:].rearrange("(k p) t -> p k t", p=128)       # (128, 4, w_in)
        outr = out[:].rearrange("(m p) t -> p m t", p=128)   # (128, 2, t_loc)

        for j in range(nt):
            c0 = j * tw
            # ---- phase A: h-GEMM + activations ----
            xt = xpool.tile([128, 4, tw], BF16)
            nc.sync.dma_start(out=xt, in_=xr[:, :, c0:c0 + tw])
            g0 = psA.tile([128, tw], F32)
            g1 = psA.tile([128, tw], F32)
            for k in range(4):
                nc.tensor.matmul(
                    g0, lhsT=w1t_sb[:, k, 0:128], rhs=xt[:, k, :],
                    start=(k == 0), stop=(k == 3))
            for k in range(4):
                nc.tensor.matmul(
                    g1, lhsT=w1t_sb[:, k, 128:256], rhs=xt[:, k, :],
                    start=(k == 0), stop=(k == 3))
            left = hpool.tile([128, tw], F32)
            right = hpool.tile([128, tw], F32)
            nc.scalar.activation(left, g0, SIG, bias=b1_sb[:, 0, :], scale=1.0)
            nc.scalar.activation(right, g1, SIG, bias=b1_sb[:, 1, :], scale=1.0)
            # b = 1 - left; alternate engines to balance ACT and DVE load
            bdst = b_buf[:, c0 + 1:c0 + 1 + tw]
            if j % 2 == 0:
                nc.scalar.activation(bdst, g0, SIG, bias=negb1t_sb, scale=-1.0)
            else:
                nc.vector.tensor_scalar(out=bdst, in0=left,
                                        scalar1=-1.0, scalar2=1.0,
                                        op0=MUL, op1=ADD)
            nc.vector.tensor_mul(a_buf[:, c0 + 1:c0 + 1 + tw], left, right)

            if j == 0:
                # Halo b *= bscale (covers cols [0, halo]; col 0 is the memset)
                nc.vector.tensor_scalar(
                    out=b_buf[:, 0:halo + 1], in0=b_buf[:, 0:halo + 1],
                    scalar1=bs_sb[:, 0:1], scalar2=None, op0=MUL)

            # ---- phase B: recurrence scan over this tile ----
            init = 0.0 if j == 0 else z_buf[:, c0 - 1:c0]
            nc.vector.tensor_tensor_scan(
                out=z_buf[:, c0:c0 + tw],
                data0=a_buf[:, c0:c0 + tw],
                data1=b_buf[:, c0:c0 + tw],
                initial=init, op0=MUL, op1=ADD)

            # ---- phase C: output GEMM + activation + store ----
            if j >= 1:
                zr = z_buf[:, c0:c0 + tw]
                o0 = psB.tile([128, tw], F32)
                o1 = psB.tile([128, tw], F32)
                nc.tensor.matmul(o0, lhsT=w2t_sb[:, 0, :], rhs=zr,
                                 start=True, stop=True)
                nc.tensor.matmul(o1, lhsT=w2t_sb[:, 1, :], rhs=zr,
                                 start=True, stop=True)
                ot = opool.tile([128, 2, tw], F32)
                nc.scalar.activation(ot[:, 0, :], o0, SIG,
                                     bias=negb2_sb[:, 0, :], scale=-1.0)
                nc.scalar.activation(ot[:, 1, :], o1, SIG,
                                     bias=negb2_sb[:, 1, :], scale=-1.0)
                nc.sync.dma_start(out=outr[:, :, c0 - halo:c0 - halo + tw], in_=ot)

    nc.finalize()
    return nc


def _make_in_maps(inputs, W1, B1, W2, B2, t_loc=T_LOC, halo=HALO, n_cores=N_CORES):
    inputs = np.asarray(inputs, dtype=np.float32)
    W1 = np.asarray(W1, dtype=np.float32)
    B1 = np.ascontiguousarray(np.asarray(B1, dtype=np.float32))
    W2 = np.asarray(W2, dtype=np.float32)
    B2 = np.asarray(B2, dtype=np.float32)

    x_bf = inputs.astype(BF16_NP)
    w1t = np.ascontiguousarray(W1.T.astype(BF16_NP))          # (512, 256)
    w2t = np.ascontiguousarray(W2.T.astype(BF16_NP))          # (128, 256)
    negb1t = np.ascontiguousarray(-B1[:N_MEM])                # (128, 1)
    negb2 = np.ascontiguousarray(-B2)                         # (256, 1)

    in_maps = []
    for i in range(n_cores):
        s = i * t_loc
        lo = s - halo
        if lo < 0:
            xs = np.concatenate(
                [np.zeros((INPUT_LEN, -lo), BF16_NP), x_bf[:, :s + t_loc]],
                axis=1)
        else:
            xs = x_bf[:, lo:s + t_loc]
        bs = np.full((N_MEM, 1), 0.0 if i == 0 else 1.0, np.float32)
        in_maps.append({
            "x": np.ascontiguousarray(xs),
            "w1t": w1t, "w2t": w2t, "b1": B1,
            "negb1t": negb1t, "negb2": negb2, "bscale": bs,
        })
    return in_maps


def _run(inputs, W1, B1, W2, B2, trace=False, **kw):
    from concourse.bass_utils import run_bass_kernel_spmd

    nc = _build_nc()
    in_maps = _make_in_maps(inputs, W1, B1, W2, B2)
    res = run_bass_kernel_spmd(nc, in_maps, list(range(N_CORES)), trace=trace, **kw)
    full = np.concatenate([r["out"] for r in res.results], axis=1)
    return full, res


def kernel(inputs, W1, B1, W2, B2):
    full, _ = _run(inputs, W1, B1, W2, B2, trace=False)
    return full.astype(np.float32, copy=False)


# revision 11
# speedup vs baseline: 2.5358x; 1.0550x over previous
"""Deductron kernel for Trainium2, 8 NeuronCores, time-sharded.

Math (matching the reference):
    h = sigmoid(W1 @ x + B1); left, right = h[:128], h[128:]
    a_t = left_t * right_t; b_t = 1 - left_t
    u_0 = 0; u_t = a_{t-1} * u_{t-1} + b_{t-1}   (z[:, t] = u_t)
    out = 1 - sigmoid(W2 @ z + B2) = sigmoid(-(W2 @ z + B2))

Sharding: the 65536-frame time axis is split into 8 chunks of 8192. Each core
also receives a 512-frame left halo. Because a_t = sigmoid(.)*sigmoid(.) < 1
and the product of 512 consecutive a's underflows to exactly 0 in fp32, the
recurrence state forgets its initial condition within the halo, so no
cross-core state exchange is needed. Core 0 has no real halo: its input is
zero-padded and a per-core input vector bscale (0 for core 0, 1 otherwise)
multiplies the halo's b values so the scan state stays exactly 0 until the
owned region starts (u_0 = 0 exactly).

The host pre-casts x to bf16 (halves input DMA) and pre-transposes the
weights into the PE's stationary layout. On-core recurrence: the Vector
engine's tensor_tensor_scan instruction computes state = a_t*state + b_t
natively along the free axis; chunk scans are chained by a 128x1 carry.
"""

import sys

for _p in ("/opt/trn_rl_repo", "/opt/pypackages"):
    if _p not in sys.path:
        sys.path.append(_p)

import numpy as np
import ml_dtypes

# Problem constants (hardcoded per contract).
INPUT_LEN = 512
N_MEM = 128  # memory dim (recurrence state width) = one partition tile
OUT_LEN = 256
T_TOTAL = 65536
N_CORES = 8
T_LOC = T_TOTAL // N_CORES  # 8192 owned frames per core
HALO = 512                  # washout halo; prod(a) over 512 steps == 0 in fp32
TW = 512                    # column tile width (one PSUM bank of fp32)
W_IN = HALO + T_LOC         # per-core input width (8704)
NT = W_IN // TW             # 17 column tiles (tile 0 is pure halo)

BF16_NP = ml_dtypes.bfloat16


def _build_nc(t_loc=T_LOC, halo=HALO, tw=TW):
    import concourse.tile as tile
    from concourse import bacc, mybir
    from contextlib import ExitStack

    F32 = mybir.dt.float32
    BF16 = mybir.dt.bfloat16
    SIG = mybir.ActivationFunctionType.Sigmoid
    MUL = mybir.AluOpType.mult
    ADD = mybir.AluOpType.add

    w_in = halo + t_loc
    nt = w_in // tw
    assert w_in % tw == 0 and halo == tw

    nc = bacc.Bacc()
    x = nc.dram_tensor("x", [INPUT_LEN, w_in], BF16, kind="ExternalInput")
    w1t = nc.dram_tensor("w1t", [INPUT_LEN, 2 * N_MEM], BF16, kind="ExternalInput")
    w2t = nc.dram_tensor("w2t", [N_MEM, OUT_LEN], BF16, kind="ExternalInput")
    b1 = nc.dram_tensor("b1", [2 * N_MEM, 1], F32, kind="ExternalInput")
    negb1t = nc.dram_tensor("negb1t", [N_MEM, 1], F32, kind="ExternalInput")
    negb2 = nc.dram_tensor("negb2", [OUT_LEN, 1], F32, kind="ExternalInput")
    bscale = nc.dram_tensor("bscale", [N_MEM, 1], F32, kind="ExternalInput")
    out = nc.dram_tensor("out", [OUT_LEN, t_loc], F32, kind="ExternalOutput")

    with ExitStack() as ctx:
        tc = ctx.enter_context(tile.TileContext(nc))
        singles = ctx.enter_context(tc.tile_pool(name="singles", bufs=1))
        xpool = ctx.enter_context(tc.tile_pool(name="xpool", bufs=6))
        hpool = ctx.enter_context(tc.tile_pool(name="hpool", bufs=4))
        opool = ctx.enter_context(tc.tile_pool(name="opool", bufs=4))
        psA = ctx.enter_context(tc.tile_pool(name="psA", bufs=2, space="PSUM"))
        psB = ctx.enter_context(tc.tile_pool(name="psB", bufs=2, space="PSUM"))

        # Persistent full-width recurrence buffers. a_buf/b_buf are written at
        # a +1 column offset (a_buf[:, p] = a at input column p-1) so the scan
        # output z[:, p] = u at column p directly.
        a_buf = singles.tile([N_MEM, w_in + 1], F32)
        b_buf = singles.tile([N_MEM, w_in + 1], F32)
        z_buf = singles.tile([N_MEM, w_in], BF16)

        # ---- weights / biases (host provides transposed layouts) ----
        w1t_sb = singles.tile([128, 4, 2 * N_MEM], BF16)
        nc.sync.dma_start(out=w1t_sb,
                          in_=w1t[:].rearrange("(k p) m -> p k m", p=128))
        w2t_sb = singles.tile([128, 2, N_MEM], BF16)
        nc.sync.dma_start(out=w2t_sb,
                          in_=w2t[:].rearrange("p (m j) -> p m j", m=2))
        b1_sb = singles.tile([128, 2, 1], F32)
        nc.sync.dma_start(out=b1_sb, in_=b1[:].rearrange("(m p) o -> p m o", p=128))
        negb1t_sb = singles.tile([128, 1], F32)
        nc.sync.dma_start(out=negb1t_sb, in_=negb1t[:])
        negb2_sb = singles.tile([128, 2, 1], F32)
        nc.sync.dma_start(out=negb2_sb,
                          in_=negb2[:].rearrange("(m p) o -> p m o", p=128))
        bs_sb = singles.tile([128, 1], F32)
        nc.sync.dma_start(out=bs_sb, in_=bscale[:])

        nc.vector.memset(a_buf[:, 0:1], 0.0)
        nc.vector.memset(b_buf[:, 0:1], 0.0)

        xr = x[:].rearrange("(k p) t -> p k t", p=128)       # (128, 4, w_in)
        outr = out[:].rearrange("(m p) t -> p m t", p=128)   # (128, 2, t_loc)

        for j in range(nt):
            c0 = j * tw
            # ---- phase A: h-GEMM + activations ----
            xt = xpool.tile([128, 4, tw], BF16)
            nc.sync.dma_start(out=xt, in_=xr[:, :, c0:c0 + tw])
            g0 = psA.tile([128, tw], F32)
            g1 = psA.tile([128, tw], F32)
            for k in range(4):
                nc.tensor.matmul(
                    g0, lhsT=w1t_sb[:, k, 0:128], rhs=xt[:, k, :],
                    start=(k == 0), stop=(k == 3))
            for k in range(4):
                nc.tensor.matmul(
                    g1, lhsT=w1t_sb[:, k, 128:256], rhs=xt[:, k, :],
                    start=(k == 0), stop=(k == 3))
            left = hpool.tile([128, tw], F32)
            right = hpool.tile([128, tw], F32)
            nc.scalar.activation(left, g0, SIG, bias=b1_sb[:, 0, :], scale=1.0)
            nc.scalar.activation(right, g1, SIG, bias=b1_sb[:, 1, :], scale=1.0)
            # b = 1 - left on DVE (keeps the Scalar engine under the DMA roof)
            bdst = b_buf[:, c0 + 1:c0 + 1 + tw]
            nc.vector.tensor_scalar(out=bdst, in0=left,
                                    scalar1=-1.0, scalar2=1.0,
                                    op0=MUL, op1=ADD)
            nc.vector.tensor_mul(a_buf[:, c0 + 1:c0 + 1 + tw], left, right)

            if j == 0:
                # Halo b *= bscale (covers cols [0, halo]; col 0 is the memset)
                nc.vector.tensor_scalar(
                    out=b_buf[:, 0:halo + 1], in0=b_buf[:, 0:halo + 1],
                    scalar1=bs_sb[:, 0:1], scalar2=None, op0=MUL)

            # ---- phase B: recurrence scan over this tile ----
            init = 0.0 if j == 0 else z_buf[:, c0 - 1:c0]
            nc.vector.tensor_tensor_scan(
                out=z_buf[:, c0:c0 + tw],
                data0=a_buf[:, c0:c0 + tw],
                data1=b_buf[:, c0:c0 + tw],
                initial=init, op0=MUL, op1=ADD)

            # ---- phase C: output GEMM + activation + store ----
            if j >= 1:
                zr = z_buf[:, c0:c0 + tw]
                o0 = psB.tile([128, tw], F32)
                o1 = psB.tile([128, tw], F32)
                nc.tensor.matmul(o0, lhsT=w2t_sb[:, 0, :], rhs=zr,
                                 start=True, stop=True)
                nc.tensor.matmul(o1, lhsT=w2t_sb[:, 1, :], rhs=zr,
                                 start=True, stop=True)
                ot = opool.tile([128, 2, tw], F32)
                nc.scalar.activation(ot[:, 0, :], o0, SIG,
                                     bias=negb2_sb[:, 0, :], scale=-1.0)
                nc.scalar.activation(ot[:, 1, :], o1, SIG,
                                     bias=negb2_sb[:, 1, :], scale=-1.0)
                nc.sync.dma_start(out=outr[:, :, c0 - halo:c0 - halo + tw], in_=ot)

    nc.finalize()
    return nc


def _make_in_maps(inputs, W1, B1, W2, B2, t_loc=T_LOC, halo=HALO, n_cores=N_CORES):
    inputs = np.asarray(inputs, dtype=np.float32)
    W1 = np.asarray(W1, dtype=np.float32)
    B1 = np.ascontiguousarray(np.asarray(B1, dtype=np.float32))
    W2 = np.asarray(W2, dtype=np.float32)
    B2 = np.asarray(B2, dtype=np.float32)

    x_bf = inputs.astype(BF16_NP)
    w1t = np.ascontiguousarray(W1.T.astype(BF16_NP))          # (512, 256)
    w2t = np.ascontiguousarray(W2.T.astype(BF16_NP))          # (128, 256)
    negb1t = np.ascontiguousarray(-B1[:N_MEM])                # (128, 1)
    negb2 = np.ascontiguousarray(-B2)                         # (256, 1)

    in_maps = []
    for i in range(n_cores):
        s = i * t_loc
        lo = s - halo
        if lo < 0:
            xs = np.concatenate(
                [np.zeros((INPUT_LEN, -lo), BF16_NP), x_bf[:, :s + t_loc]],
                axis=1)
        else:
            xs = x_bf[:, lo:s + t_loc]
        bs = np.full((N_MEM, 1), 0.0 if i == 0 else 1.0, np.float32)
        in_maps.append({
            "x": np.ascontiguousarray(xs),
            "w1t": w1t, "w2t": w2t, "b1": B1,
            "negb1t": negb1t, "negb2": negb2, "bscale": bs,
        })
    return in_maps


def _run(inputs, W1, B1, W2, B2, trace=False, **kw):
    from concourse.bass_utils import run_bass_kernel_spmd

    nc = _build_nc()
    in_maps = _make_in_maps(inputs, W1, B1, W2, B2)
    res = run_bass_kernel_spmd(nc, in_maps, list(range(N_CORES)), trace=trace, **kw)
    full = np.concatenate([r["out"] for r in res.results], axis=1)
    return full, res


def kernel(inputs, W1, B1, W2, B2):
    full, _ = _run(inputs, W1, B1, W2, B2, trace=False)
    return full.astype(np.float32, copy=False)


# revision 13
# speedup vs baseline: 2.8417x; 1.1206x over previous
"""Deductron kernel for Trainium2, 8 NeuronCores, time-sharded.

Math (matching the reference):
    h = sigmoid(W1 @ x + B1); left, right = h[:128], h[128:]
    a_t = left_t * right_t; b_t = 1 - left_t
    u_0 = 0; u_t = a_{t-1} * u_{t-1} + b_{t-1}   (z[:, t] = u_t)
    out = 1 - sigmoid(W2 @ z + B2) = sigmoid(-(W2 @ z + B2))

Sharding: the 65536-frame time axis is split into 8 chunks of 8192. Each core
also receives a 512-frame left halo. Because a_t = sigmoid(.)*sigmoid(.) < 1
and the product of 512 consecutive a's underflows to exactly 0 in fp32, the
recurrence state forgets its initial condition within the halo, so no
cross-core state exchange is needed. Core 0 has no real halo: its input is
zero-padded and a per-core input vector bscale (0 for core 0, 1 otherwise)
multiplies the halo's b values so the scan state stays exactly 0 until the
owned region starts (u_0 = 0 exactly).

The host pre-casts x to bf16 (halves input DMA) and pre-transposes the
weights into the PE's stationary layout. On-core recurrence: the Vector
engine's tensor_tensor_scan instruction computes state = a_t*state + b_t
natively along the free axis; chunk scans are chained by a 128x1 carry.
"""

import sys

for _p in ("/opt/trn_rl_repo", "/opt/pypackages"):
    if _p not in sys.path:
        sys.path.append(_p)

import numpy as np
import ml_dtypes

# Problem constants (hardcoded per contract).
INPUT_LEN = 512
N_MEM = 128  # memory dim (recurrence state width) = one partition tile
OUT_LEN = 256
T_TOTAL = 65536
N_CORES = 8
T_LOC = T_TOTAL // N_CORES  # 8192 owned frames per core
HALO = 512                  # washout halo; prod(a) over 512 steps == 0 in fp32
TW = 512                    # column tile width (one PSUM bank of fp32)
W_IN = HALO + T_LOC         # per-core input width (8704)
NT = W_IN // TW             # 17 column tiles (tile 0 is pure halo)

BF16_NP = ml_dtypes.bfloat16


def _build_nc(t_loc=T_LOC, halo=HALO, tw=TW):
    import concourse.tile as tile
    from concourse import bacc, mybir
    from contextlib import ExitStack

    F32 = mybir.dt.float32
    BF16 = mybir.dt.bfloat16
    SIG = mybir.ActivationFunctionType.Sigmoid
    MUL = mybir.AluOpType.mult
    ADD = mybir.AluOpType.add

    w_in = halo + t_loc
    nt = w_in // tw
    assert w_in % tw == 0 and halo == tw

    nc = bacc.Bacc()
    x = nc.dram_tensor("x", [INPUT_LEN, w_in], BF16, kind="ExternalInput")
    w1t = nc.dram_tensor("w1t", [INPUT_LEN, 2 * N_MEM], BF16, kind="ExternalInput")
    w2t = nc.dram_tensor("w2t", [N_MEM, OUT_LEN], BF16, kind="ExternalInput")
    b1 = nc.dram_tensor("b1", [2 * N_MEM, 1], F32, kind="ExternalInput")
    negb1t = nc.dram_tensor("negb1t", [N_MEM, 1], F32, kind="ExternalInput")
    negb2 = nc.dram_tensor("negb2", [OUT_LEN, 1], F32, kind="ExternalInput")
    bscale = nc.dram_tensor("bscale", [N_MEM, 1], F32, kind="ExternalInput")
    out = nc.dram_tensor("out", [OUT_LEN, t_loc], F32, kind="ExternalOutput")

    with ExitStack() as ctx:
        tc = ctx.enter_context(tile.TileContext(nc))
        singles = ctx.enter_context(tc.tile_pool(name="singles", bufs=1))
        xpool = ctx.enter_context(tc.tile_pool(name="xpool", bufs=6))
        hpool = ctx.enter_context(tc.tile_pool(name="hpool", bufs=4))
        opool = ctx.enter_context(tc.tile_pool(name="opool", bufs=4))
        psA = ctx.enter_context(tc.tile_pool(name="psA", bufs=2, space="PSUM"))
        psB = ctx.enter_context(tc.tile_pool(name="psB", bufs=2, space="PSUM"))

        # Persistent full-width recurrence buffers. a_buf/b_buf are written at
        # a +1 column offset (a_buf[:, p] = a at input column p-1) so the scan
        # output z[:, p] = u at column p directly.
        a_buf = singles.tile([N_MEM, w_in + 1], F32)
        b_buf = singles.tile([N_MEM, w_in + 1], F32)
        z_buf = singles.tile([N_MEM, w_in], BF16)

        # ---- weights / biases (host provides transposed layouts) ----
        w1t_sb = singles.tile([128, 4, 2 * N_MEM], BF16)
        nc.sync.dma_start(out=w1t_sb,
                          in_=w1t[:].rearrange("(k p) m -> p k m", p=128))
        w2t_sb = singles.tile([128, 2, N_MEM], BF16)
        nc.sync.dma_start(out=w2t_sb,
                          in_=w2t[:].rearrange("p (m j) -> p m j", m=2))
        b1_sb = singles.tile([128, 2, 1], F32)
        nc.sync.dma_start(out=b1_sb, in_=b1[:].rearrange("(m p) o -> p m o", p=128))
        negb1t_sb = singles.tile([128, 1], F32)
        nc.sync.dma_start(out=negb1t_sb, in_=negb1t[:])
        negb2_sb = singles.tile([128, 2, 1], F32)
        nc.sync.dma_start(out=negb2_sb,
                          in_=negb2[:].rearrange("(m p) o -> p m o", p=128))
        bs_sb = singles.tile([128, 1], F32)
        nc.sync.dma_start(out=bs_sb, in_=bscale[:])

        nc.vector.memset(a_buf[:, 0:1], 0.0)
        nc.vector.memset(b_buf[:, 0:1], 0.0)

        xr = x[:].rearrange("(k p) t -> p k t", p=128)       # (128, 4, w_in)
        outr = out[:].rearrange("(m p) t -> p m t", p=128)   # (128, 2, t_loc)

        def phase_c(j):
            # output GEMM + activation + store for z tile j (j >= 1)
            c0 = j * tw
            zr = z_buf[:, c0:c0 + tw]
            o0 = psB.tile([128, tw], F32)
            o1 = psB.tile([128, tw], F32)
            nc.tensor.matmul(o0, lhsT=w2t_sb[:, 0, :], rhs=zr,
                             start=True, stop=True)
            nc.tensor.matmul(o1, lhsT=w2t_sb[:, 1, :], rhs=zr,
                             start=True, stop=True)
            ot = opool.tile([128, 2, tw], F32)
            nc.scalar.activation(ot[:, 0, :], o0, SIG,
                                 bias=negb2_sb[:, 0, :], scale=-1.0)
            nc.scalar.activation(ot[:, 1, :], o1, SIG,
                                 bias=negb2_sb[:, 1, :], scale=-1.0)
            nc.sync.dma_start(out=outr[:, :, c0 - halo:c0 - halo + tw], in_=ot)

        # Phase C is emitted DELAY tiles behind phases A/B: the PE stream is
        # in-order, so a GEMM2 queued right after scan j would stall the PE
        # (and everything downstream) on the serial scan spine. The delay
        # keeps the PE fed with work whose inputs are already resolved.
        DELAY = 3
        for j in range(nt):
            c0 = j * tw
            # ---- phase A: h-GEMM + activations ----
            xt = xpool.tile([128, 4, tw], BF16)
            nc.sync.dma_start(out=xt, in_=xr[:, :, c0:c0 + tw])
            g0 = psA.tile([128, tw], F32)
            g1 = psA.tile([128, tw], F32)
            for k in range(4):
                nc.tensor.matmul(
                    g0, lhsT=w1t_sb[:, k, 0:128], rhs=xt[:, k, :],
                    start=(k == 0), stop=(k == 3))
            for k in range(4):
                nc.tensor.matmul(
                    g1, lhsT=w1t_sb[:, k, 128:256], rhs=xt[:, k, :],
                    start=(k == 0), stop=(k == 3))
            left = hpool.tile([128, tw], F32)
            right = hpool.tile([128, tw], F32)
            nc.scalar.activation(left, g0, SIG, bias=b1_sb[:, 0, :], scale=1.0)
            nc.scalar.activation(right, g1, SIG, bias=b1_sb[:, 1, :], scale=1.0)
            # b = 1 - left on DVE (keeps the Scalar engine under the DMA roof)
            bdst = b_buf[:, c0 + 1:c0 + 1 + tw]
            nc.vector.tensor_scalar(out=bdst, in0=left,
                                    scalar1=-1.0, scalar2=1.0,
                                    op0=MUL, op1=ADD)
            nc.vector.tensor_mul(a_buf[:, c0 + 1:c0 + 1 + tw], left, right)

            if j == 0:
                # Halo b *= bscale (covers cols [0, halo]; col 0 is the memset)
                nc.vector.tensor_scalar(
                    out=b_buf[:, 0:halo + 1], in0=b_buf[:, 0:halo + 1],
                    scalar1=bs_sb[:, 0:1], scalar2=None, op0=MUL)

            # ---- phase B: recurrence scan over this tile ----
            init = 0.0 if j == 0 else z_buf[:, c0 - 1:c0]
            nc.vector.tensor_tensor_scan(
                out=z_buf[:, c0:c0 + tw],
                data0=a_buf[:, c0:c0 + tw],
                data1=b_buf[:, c0:c0 + tw],
                initial=init, op0=MUL, op1=ADD)

            # ---- phase C, delayed ----
            if j - DELAY >= 1:
                phase_c(j - DELAY)

        for j in range(max(1, nt - DELAY), nt):
            phase_c(j)

    nc.finalize()
    return nc


def _make_in_maps(inputs, W1, B1, W2, B2, t_loc=T_LOC, halo=HALO, n_cores=N_CORES):
    inputs = np.asarray(inputs, dtype=np.float32)
    W1 = np.asarray(W1, dtype=np.float32)
    B1 = np.ascontiguousarray(np.asarray(B1, dtype=np.float32))
    W2 = np.asarray(W2, dtype=np.float32)
    B2 = np.asarray(B2, dtype=np.float32)

    x_bf = inputs.astype(BF16_NP)
    w1t = np.ascontiguousarray(W1.T.astype(BF16_NP))          # (512, 256)
    w2t = np.ascontiguousarray(W2.T.astype(BF16_NP))          # (128, 256)
    negb1t = np.ascontiguousarray(-B1[:N_MEM])                # (128, 1)
    negb2 = np.ascontiguousarray(-B2)                         # (256, 1)

    in_maps = []
    for i in range(n_cores):
        s = i * t_loc
        lo = s - halo
        if lo < 0:
            xs = np.concatenate(
                [np.zeros((INPUT_LEN, -lo), BF16_NP), x_bf[:, :s + t_loc]],
                axis=1)
        else:
            xs = x_bf[:, lo:s + t_loc]
        bs = np.full((N_MEM, 1), 0.0 if i == 0 else 1.0, np.float32)
        in_maps.append({
            "x": np.ascontiguousarray(xs),
            "w1t": w1t, "w2t": w2t, "b1": B1,
            "negb1t": negb1t, "negb2": negb2, "bscale": bs,
        })
    return in_maps


def _run(inputs, W1, B1, W2, B2, trace=False, **kw):
    from concourse.bass_utils import run_bass_kernel_spmd

    nc = _build_nc()
    in_maps = _make_in_maps(inputs, W1, B1, W2, B2)
    res = run_bass_kernel_spmd(nc, in_maps, list(range(N_CORES)), trace=trace, **kw)
    full = np.concatenate([r["out"] for r in res.results], axis=1)
    return full, res


def kernel(inputs, W1, B1, W2, B2):
    full, _ = _run(inputs, W1, B1, W2, B2, trace=False)
    return full.astype(np.float32, copy=False)
